# revision 13
# baseline (speedup 1.0000x reference)
"""Trainium2 Bass kernel for nn_DegModel (EDSR-style degradation backbone +
per-pixel KPN), distributed over 8 NeuronCores.

Sharding: one core per (batch, image-half): core i -> batch i//2, half i%2.
Each core runs the whole backbone locally on its 64-row half plus a 17-row
recomputed halo, so no collectives are needed. Bottom halves are processed
vertically flipped (host flips z and the dy axis of the conv weights, both
per-core input data), which makes the on-device geometry identical for all
cores. The only cross-core quantity — the global mean of the predicted noise
channel — is reduced on host from per-core partial sums.

Feature maps live in SBUF as [128 partitions, J slots, 130] with partition
p = channel + 64*parity and the odd-row half skewed one slot down:
lower[c, j] = F[c, 2j], upper[c, j] = F[c, 2j-1]. With this skew a 3x3 conv
over an 8-row output block is exactly 6 full K=128 x M=128 float32r matmuls
(2 per kernel column dx) into one [128, 4, 128] PSUM bank: M columns 0:64
produce the even output rows, 64:128 the odd rows.

conv_out (1x1) + softmax + the 21x21 KPN run per output row in
pixel-partition layout: feat[:, slot, 1:129] is the stationary operand
(M = 128 pixels) against moving w_out [128, 442], landing logits as
[pixel, channel]; exp / reduce / divide are then free-dim ops, and the
softmax normalization is folded to after the KPN sum
(y = sum(patch * exp) / sum(exp)).
"""

import sys

sys.path.insert(0, "/opt/trn_rl_repo")

import numpy as np

import concourse.bass as bass
import concourse.tile as tile
from concourse import mybir
from concourse.bass_utils import run_bass_kernel_spmd

KSIZE = 21
NF = 64
NB = 8
IN_NC = 3
B, H, W = 4, 512, 512
h = w = 128
NCH = KSIZE * KSIZE + 1  # 442

N_CORES = 8
J = 44    # feature-buffer slots (2 image rows per slot)
X = 130   # 128 cols + 2 zero pad cols
NMID = 2 * NB
XPW = 532  # padded x row length (512 + 2*10)

_cache = {}


def _enable_ldw_opt():
    import concourse.bass_utils as _bu
    if getattr(_bu, "_ldw_opt_patched", False):
        return
    _orig = _bu.run_command

    def _patched(cmd, **kw):
        if isinstance(cmd, list):
            cmd = ["--enable-ldw-opt=true" if c == "--enable-ldw-opt=false"
                   else c for c in cmd]
        return _orig(cmd, **kw)

    _bu.run_command = _patched
    _bu._ldw_opt_patched = True


def _legalize_waits(nc):
    """This walrus build rejects >1 sync wait per instruction; move extra
    waits onto same-engine NOPs inserted immediately before (engines are
    in-order, so semantics are preserved)."""
    for fn in nc.m.functions:
        for blk in fn.blocks:
            out, changed = [], False
            for inst in blk.instructions:
                si = inst.sync_info
                if si is not None and len(si.on_wait) > 1:
                    waits = list(si.on_wait)
                    for wt in waits[:-1]:
                        nop = mybir.InstNoOp(
                            name=nc.get_next_instruction_name(),
                            ins=[], outs=[], engine=inst.engine)
                        nop.sync_info = mybir.SyncInfo(on_wait=[wt], on_update=[])
                        out.append(nop)
                        changed = True
                    inst.sync_info = mybir.SyncInfo(
                        on_wait=[waits[-1]], on_update=list(si.on_update))
                out.append(inst)
            if changed:
                blk.instructions = out


def _build_nc(bias2_zero, bout_zero):
    f32 = mybir.dt.float32
    f32r = mybir.dt.float32r
    nc = bass.Bass()

    zg_e = nc.dram_tensor("zg_e", [IN_NC, 41, 128], f32r, kind="ExternalInput")
    zg_o = nc.dram_tensor("zg_o", [IN_NC, 41, 128], f32r, kind="ExternalInput")
    wl1_in = nc.dram_tensor("wl1_in", [128, 3, 128], f32r, kind="ExternalInput")
    wl2_in = nc.dram_tensor("wl2_in", [128, 3, 128], f32r, kind="ExternalInput")
    wl1_mid = nc.dram_tensor("wl1_mid", [NMID, 128, 3, 128], f32r,
                             kind="ExternalInput")
    wl2_mid = nc.dram_tensor("wl2_mid", [NMID, 128, 3, 128], f32r,
                             kind="ExternalInput")
    wout_lo = nc.dram_tensor("wout_lo", [128, NCH], f32r, kind="ExternalInput")
    wout_hi = nc.dram_tensor("wout_hi", [128, NCH], f32r, kind="ExternalInput")
    biases = nc.dram_tensor("biases", [NMID + 1, 128, 1], f32,
                            kind="ExternalInput")
    bout_r = nc.dram_tensor("bout_r", [1, NCH], f32r, kind="ExternalInput")
    ones_r = nc.dram_tensor("ones_r", [1, 128], f32r, kind="ExternalInput")
    # per-(channel,row) expanded KPN patch windows, per-partition contiguous
    # (882B per partition -> one descriptor each, 128 per DMA). bf16 for DVE
    # 2x-mode multiplies.
    bf16 = mybir.dt.bfloat16
    xw = nc.dram_tensor("xw", [IN_NC, 64, 128, KSIZE, KSIZE], bf16,
                        kind="ExternalInput")

    ydev = nc.dram_tensor("ydev", [128, IN_NC, 64], f32, kind="ExternalOutput")
    nsdev = nc.dram_tensor("nsdev", [128, 64], f32, kind="ExternalOutput")

    with tile.TileContext(nc) as tc:
        wpool = tc.alloc_tile_pool(name="w", bufs=1)
        gpool = tc.alloc_tile_pool(name="g", bufs=1)
        wmpool = tc.alloc_tile_pool(name="wmid", bufs=3)
        tpool = tc.alloc_tile_pool(name="rtmp", bufs=3)
        ppool = tc.alloc_tile_pool(name="patch", bufs=8)
        epool = tc.alloc_tile_pool(name="exp", bufs=3)
        spool = tc.alloc_tile_pool(name="small", bufs=4)
        psum = tc.alloc_tile_pool(name="ps", bufs=5, space="PSUM")
        psum_o = tc.alloc_tile_pool(name="pso", bufs=2, space="PSUM")

        l1_in = wpool.tile([128, 3, 128], f32r)
        l2_in = wpool.tile([128, 3, 128], f32r)
        wo_lo = wpool.tile([128, NCH], f32r)
        wo_hi = wpool.tile([128, NCH], f32r)
        bias_t = wpool.tile([128, NMID + 1], f32)
        bo_t = wpool.tile([1, NCH], f32r)
        ones_t = wpool.tile([1, 128], f32r)
        nc.sync.dma_start(out=l1_in, in_=wl1_in[:])
        nc.sync.dma_start(out=l2_in, in_=wl2_in[:])
        nc.sync.dma_start(out=wo_lo, in_=wout_lo[:])
        nc.sync.dma_start(out=wo_hi, in_=wout_hi[:])
        nc.sync.dma_start(out=bias_t,
                          in_=biases[:].rearrange("l p one -> p (l one)"))
        nc.sync.dma_start(out=bo_t, in_=bout_r[:])
        nc.sync.dma_start(out=ones_t, in_=ones_r[:])

        g_z = gpool.tile([128, J, X], f32r)
        feat = gpool.tile([128, J, X], f32r)
        t1 = gpool.tile([128, J, X], f32r)
        nc.vector.memset(g_z[:].bitcast(mybir.dt.float32), 0.0)
        nc.vector.memset(feat[:].bitcast(mybir.dt.float32), 0.0)
        nc.vector.memset(t1[:].bitcast(mybir.dt.float32), 0.0)

        # z rows (shard-local, 0..80): even row r -> partitions 0:3 slot
        # r//2+1; odd row r -> partitions 64:67 slot (r+1)//2+1.
        # Host packs zg_e = even rows 0..80 (slots 1..41), zg_o = odd rows
        # 1..79 at slots 2..41 (zg_o[0] stays zero -> slot 1 zero = row -1).
        nc.sync.dma_start(out=g_z[0:IN_NC, 1:42, 1:129], in_=zg_e[:])
        nc.sync.dma_start(out=g_z[64:64 + IN_NC, 1:42, 1:129], in_=zg_o[:])

        relu = mybir.ActivationFunctionType.Relu
        ident = mybir.ActivationFunctionType.Identity

        def conv(src, dst, l1, l2, bias_col, func, k_halo, residual):
            # output region: shard-local rows 0 .. 63 + k_halo -> slots 1..hi
            hi = (64 + k_halo) // 2 + 1      # top slot of even output rows
            blocks = [(s, min(4, hi - s + 1)) for s in range(1, hi + 1, 4)]
            # weight-major inside groups of 5 blocks: consecutive matmuls
            # share the stationary operand so walrus ldw-opt dedups the
            # (serialized, non-overlapping) LDWEIGHTS streams.
            for g0 in range(0, len(blocks), 5):
                grp = blocks[g0:g0 + 5]
                tiles = [psum.tile([128, 4, 128], f32, tag="convps",
                                   name=f"cps_{g0}_{i}")
                         for i in range(len(grp))]
                for wi in range(6):
                    dx, phase = wi % 3, wi // 3
                    wt = (l1 if phase == 0 else l2)[:, dx]
                    for (s0, mc), P in zip(grp, tiles):
                        o = s0 + phase
                        nc.tensor.matmul(
                            P[:, 0:mc], wt,
                            src[0:128, o:o + mc, dx:dx + 128],
                            start=(wi == 0), stop=(wi == 5))
                for (s0, mc), P in zip(grp, tiles):
                    if residual is None:
                        nc.scalar.activation(
                            out=dst[0:64, s0:s0 + mc, 1:129],
                            in_=P[0:64, 0:mc],
                            func=func, bias=bias_col[0:64], scale=1.0)
                        nc.scalar.activation(
                            out=dst[64:128, s0 + 1:s0 + 1 + mc, 1:129],
                            in_=P[64:128, 0:mc],
                            func=func, bias=bias_col[64:128], scale=1.0)
                    else:
                        # evacuate via ACT (bias folded), accumulate the
                        # residual on GpSimd (SBUF-only engine, otherwise
                        # idle) to keep DVE off the critical path
                        tmp = tpool.tile([128, 4, 128], f32, tag="rtmp")
                        nc.scalar.activation(
                            out=tmp[:, 0:mc], in_=P[:, 0:mc], func=ident,
                            bias=0.0, scale=1.0)
                        if not bias2_zero:
                            nc.vector.tensor_scalar(
                                out=tmp[:, 0:mc], in0=tmp[:, 0:mc],
                                scalar1=bias_col, scalar2=None,
                                op0=mybir.AluOpType.add)
                        nc.gpsimd.tensor_add(
                            out=dst[0:64, s0:s0 + mc, 1:129],
                            in0=tmp[0:64, 0:mc],
                            in1=residual[0:64, s0:s0 + mc, 1:129])
                        nc.gpsimd.tensor_add(
                            out=dst[64:128, s0 + 1:s0 + 1 + mc, 1:129],
                            in0=tmp[64:128, 0:mc],
                            in1=residual[64:128, s0 + 1:s0 + 1 + mc, 1:129])

        conv(g_z, feat, l1_in, l2_in, bias_t[:, 0:1], ident, 16, None)
        for rb in range(NB):
            la, lb = 2 * rb, 2 * rb + 1
            w1a = wmpool.tile([128, 3, 128], f32r, tag="w1")
            w2a = wmpool.tile([128, 3, 128], f32r, tag="w2")
            nc.sync.dma_start(out=w1a, in_=wl1_mid[la])
            nc.sync.dma_start(out=w2a, in_=wl2_mid[la])
            conv(feat, t1, w1a, w2a,
                 bias_t[:, 1 + la:2 + la], relu, 15 - 2 * rb, None)
            w1b = wmpool.tile([128, 3, 128], f32r, tag="w1")
            w2b = wmpool.tile([128, 3, 128], f32r, tag="w2")
            nc.sync.dma_start(out=w1b, in_=wl1_mid[lb])
            nc.sync.dma_start(out=w2b, in_=wl2_mid[lb])
            conv(t1, feat, w1b, w2b,
                 bias_t[:, 1 + lb:2 + lb], ident, 14 - 2 * rb, feat)

        yacc = spool.tile([128, IN_NC, 64], f32, tag="yacc")
        nsacc = spool.tile([128, 64], f32, tag="nsacc")

        for yl in range(64):
            if yl % 2 == 0:
                slot, wsel = yl // 2 + 1, wo_lo
            else:
                slot, wsel = (yl + 1) // 2 + 1, wo_hi
            Po = psum_o.tile([128, NCH], f32, tag="pout")
            nc.tensor.matmul(Po, feat[:, slot, 1:129], wsel,
                             start=True, stop=bout_zero)
            if not bout_zero:
                nc.tensor.matmul(Po, ones_t, bo_t, start=False, stop=True)
            ex = epool.tile([128, NCH], bf16, tag="ex")
            nc.scalar.activation(out=ex, in_=Po,
                                 func=mybir.ActivationFunctionType.Exp,
                                 scale=1.0)
            ssum = spool.tile([128, 1], f32, tag="ssum")
            nc.vector.reduce_sum(out=ssum, in_=ex, axis=mybir.AxisListType.X)
            rcp = spool.tile([128, 1], f32, tag="rcp")
            nc.vector.reciprocal(out=rcp, in_=ssum)
            exv = ex[:, 0:KSIZE * KSIZE].rearrange(
                "p (a b) -> p a b", a=KSIZE)
            dma_engines = [nc.sync, nc.sync, nc.sync]
            for c in range(IN_NC):
                patch = ppool.tile([128, KSIZE, KSIZE], bf16, tag="patch")
                dma_engines[c].dma_start(out=patch, in_=xw[c, yl])
                prod = epool.tile([128, KSIZE, KSIZE], bf16, tag="prod")
                pc = spool.tile([128, 1], f32, tag="pc")
                nc.vector.tensor_mul(out=prod, in0=exv, in1=patch)
                nc.vector.reduce_sum(
                    out=pc, in_=prod.rearrange("p a b -> p (a b)"),
                    axis=mybir.AxisListType.X)
                nc.vector.tensor_mul(out=yacc[:, c, yl:yl + 1], in0=pc,
                                     in1=rcp)
            nc.vector.tensor_mul(out=nsacc[:, yl:yl + 1],
                                 in0=ex[:, NCH - 1:NCH], in1=rcp)

        nc.sync.dma_start(out=ydev[:], in_=yacc)
        nc.sync.dma_start(out=nsdev[:], in_=nsacc)

        for p in (psum_o, psum, spool, epool, ppool, tpool, wmpool, gpool,
                  wpool):
            p.release()

    _legalize_waits(nc)
    return nc


def _stack_l1l2(Wl):
    # Wl [64o, ic, 3, 3] -> L1, L2 [128, 3, 128]
    ic = Wl.shape[1]
    L1 = np.zeros((128, 3, 128), np.float32)
    L2 = np.zeros((128, 3, 128), np.float32)
    for dx in range(3):
        L1[0:ic, dx, 0:64] = Wl[:, :, 1, dx].T
        L1[64:64 + ic, dx, 0:64] = Wl[:, :, 0, dx].T
        L1[0:ic, dx, 64:128] = Wl[:, :, 0, dx].T
        L2[64:64 + ic, dx, 0:64] = Wl[:, :, 2, dx].T
        L2[0:ic, dx, 64:128] = Wl[:, :, 2, dx].T
        L2[64:64 + ic, dx, 64:128] = Wl[:, :, 1, dx].T
    return L1, L2


def _prep_weights(w_in, w1s, w2s, w_out, flip):
    if flip:
        w_in = w_in[:, :, ::-1, :]
        w1s = w1s[:, :, :, ::-1, :]
        w2s = w2s[:, :, :, ::-1, :]
    l1_in, l2_in = _stack_l1l2(w_in)
    L1m = np.zeros((NMID, 128, 3, 128), np.float32)
    L2m = np.zeros((NMID, 128, 3, 128), np.float32)
    for rb in range(NB):
        L1m[2 * rb], L2m[2 * rb] = _stack_l1l2(w1s[rb])
        L1m[2 * rb + 1], L2m[2 * rb + 1] = _stack_l1l2(w2s[rb])
    wo = w_out[:, :, 0, 0]  # [442, 64]
    wlo = np.zeros((128, NCH), np.float32)
    whi = np.zeros((128, NCH), np.float32)
    wlo[0:64] = wo.T
    whi[64:128] = wo.T
    return l1_in, l2_in, L1m, L2m, wlo, whi


def kernel(x, z, eps, w_in, b_in, w1s, b1s, w2s, b2s, w_out, b_out):
    x = np.ascontiguousarray(np.asarray(x, np.float32))
    z = np.asarray(z, np.float32)
    eps = np.asarray(eps, np.float32)
    w_in = np.asarray(w_in, np.float32)
    b_in = np.asarray(b_in, np.float32)
    w1s = np.asarray(w1s, np.float32)
    b1s = np.asarray(b1s, np.float32)
    w2s = np.asarray(w2s, np.float32)
    b2s = np.asarray(b2s, np.float32)
    w_out = np.asarray(w_out, np.float32)
    b_out = np.asarray(b_out, np.float32)

    bias2_zero = bool(np.all(b2s == 0))
    bout_zero = bool(np.all(b_out == 0))
    _enable_ldw_opt()
    key = (bias2_zero, bout_zero)
    if key not in _cache:
        _cache[key] = _build_nc(bias2_zero, bout_zero)
    nc = _cache[key]

    weights = {}
    for flip in (False, True):
        l1_in, l2_in, L1m, L2m, wlo, whi = _prep_weights(
            w_in, w1s, w2s, w_out, flip)
        weights[flip] = (l1_in, l2_in, L1m, L2m, wlo, whi)

    biases = np.zeros((NMID + 1, 128, 1), np.float32)
    biases[0, 0:64, 0] = b_in
    biases[0, 64:128, 0] = b_in
    for rb in range(NB):
        biases[1 + 2 * rb, 0:64, 0] = b1s[rb]
        biases[1 + 2 * rb, 64:128, 0] = b1s[rb]
        biases[2 + 2 * rb, 0:64, 0] = b2s[rb]
        biases[2 + 2 * rb, 64:128, 0] = b2s[rb]
    bout_row = np.ascontiguousarray(b_out.reshape(1, NCH))
    ones_row = np.ones((1, 128), np.float32)

    # padded x (vertical dim only logical; we slice rows directly)
    in_maps = []
    for core in range(N_CORES):
        b, half = core // 2, core % 2
        flip = half == 1
        # shard-local z rows 0..80: top zl[r] = z[b, r]; bottom z flipped
        zl = z[b] if not flip else z[b, :, ::-1]
        zg_e = np.zeros((IN_NC, 41, 128), np.float32)
        zg_o = np.zeros((IN_NC, 41, 128), np.float32)
        zg_e[:, 0:41] = zl[:, 0:81:2]          # rows 0,2,..,80 -> slots 1..41
        zg_o[:, 1:41] = zl[:, 1:80:2]          # rows 1,3,..,79 -> slots 2..41
        # KPN patch windows, fully expanded per output row:
        # xw[c, yl, x0, t, u] = xp[c, 4*y0(yl) + t, 4*x0 + u] with
        # y0 = yl (top) or 127 - yl (bottom flipped), xp = x padded by 10.
        import ml_dtypes
        xp = np.zeros((IN_NC, H + 2 * 10, W + 2 * 10), dtype=ml_dtypes.bfloat16)
        xp[:, 10:10 + H, 10:10 + W] = x[b]
        y0s = np.arange(64) if not flip else (127 - np.arange(64))
        ridx = (4 * y0s)[:, None] + np.arange(KSIZE)[None, :]   # [64, 21]
        cols = 4 * np.arange(128)[:, None] + np.arange(KSIZE)[None, :]
        sub = xp[:, ridx]                 # [3, 64, 21, 532]
        sub = sub[:, :, :, cols]          # [3, 64, 21, 128, 21]
        xw_arr = np.ascontiguousarray(np.transpose(sub, (0, 1, 3, 2, 4)))
        l1_in, l2_in, L1m, L2m, wlo, whi = weights[flip]
        in_maps.append({
            "zg_e": zg_e, "zg_o": zg_o,
            "wl1_in": l1_in, "wl2_in": l2_in,
            "wl1_mid": L1m, "wl2_mid": L2m,
            "wout_lo": wlo, "wout_hi": whi,
            "biases": biases, "bout_r": bout_row, "ones_r": ones_row,
            "xw": xw_arr,
        })

    trace = bool(globals().get("TRACE", False))
    res = run_bass_kernel_spmd(nc, in_maps, core_ids=list(range(N_CORES)),
                               trace=trace)
    globals()["_last_result"] = res

    out = np.zeros((B, IN_NC, h, w), np.float32)
    for bb in range(B):
        ns_sum = (float(res.results[2 * bb]["nsdev"].sum())
                  + float(res.results[2 * bb + 1]["nsdev"].sum()))
        mean_ns = ns_sum / (h * w)
        for half in range(2):
            ydev = res.results[2 * bb + half]["ydev"]  # [128, 3, 64]
            yt = np.transpose(ydev, (1, 2, 0))         # [3, 64, 128]
            if half == 0:
                out[bb, :, 0:64, :] = yt
            else:
                out[bb, :, 64:128, :] = yt[:, ::-1, :]
        out[bb] += mean_ns * eps[bb]
    return out


# revision 14
# speedup vs baseline: 1.1704x; 1.1704x over previous
"""Trainium2 Bass kernel for nn_DegModel (EDSR-style degradation backbone +
per-pixel KPN), distributed over 8 NeuronCores.

Sharding: one core per (batch, image-half): core i -> batch i//2, half i%2.
Each core runs the whole backbone locally on its 64-row half plus a 17-row
recomputed halo, so no collectives are needed. Bottom halves are processed
vertically flipped (host flips z and the dy axis of the conv weights, both
per-core input data), which makes the on-device geometry identical for all
cores. The only cross-core quantity — the global mean of the predicted noise
channel — is reduced on host from per-core partial sums.

Feature maps live in SBUF as [128 partitions, J slots, 130] with partition
p = channel + 64*parity and the odd-row half skewed one slot down:
lower[c, j] = F[c, 2j], upper[c, j] = F[c, 2j-1]. With this skew a 3x3 conv
over an 8-row output block is exactly 6 full K=128 x M=128 float32r matmuls
(2 per kernel column dx) into one [128, 4, 128] PSUM bank: M columns 0:64
produce the even output rows, 64:128 the odd rows.

conv_out (1x1) + softmax + the 21x21 KPN run per output row in
pixel-partition layout: feat[:, slot, 1:129] is the stationary operand
(M = 128 pixels) against moving w_out [128, 442], landing logits as
[pixel, channel]; exp / reduce / divide are then free-dim ops, and the
softmax normalization is folded to after the KPN sum
(y = sum(patch * exp) / sum(exp)).
"""

import sys

sys.path.insert(0, "/opt/trn_rl_repo")

import numpy as np

import concourse.bass as bass
import concourse.tile as tile
from concourse import mybir
from concourse.bass_utils import run_bass_kernel_spmd

KSIZE = 21
NF = 64
NB = 8
IN_NC = 3
B, H, W = 4, 512, 512
h = w = 128
NCH = KSIZE * KSIZE + 1  # 442

N_CORES = 8
J = 44    # feature-buffer slots (2 image rows per slot)
X = 130   # 128 cols + 2 zero pad cols
NMID = 2 * NB
XPW = 532  # padded x row length (512 + 2*10)

_cache = {}


def _enable_ldw_opt():
    import concourse.bass_utils as _bu
    if getattr(_bu, "_ldw_opt_patched", False):
        return
    _orig = _bu.run_command

    def _patched(cmd, **kw):
        if isinstance(cmd, list):
            cmd = ["--enable-ldw-opt=true" if c == "--enable-ldw-opt=false"
                   else c for c in cmd]
        return _orig(cmd, **kw)

    _bu.run_command = _patched
    _bu._ldw_opt_patched = True


def _legalize_waits(nc):
    """This walrus build rejects >1 sync wait per instruction; move extra
    waits onto same-engine NOPs inserted immediately before (engines are
    in-order, so semantics are preserved)."""
    for fn in nc.m.functions:
        for blk in fn.blocks:
            out, changed = [], False
            for inst in blk.instructions:
                si = inst.sync_info
                if si is not None and len(si.on_wait) > 1:
                    waits = list(si.on_wait)
                    for wt in waits[:-1]:
                        nop = mybir.InstNoOp(
                            name=nc.get_next_instruction_name(),
                            ins=[], outs=[], engine=inst.engine)
                        nop.sync_info = mybir.SyncInfo(on_wait=[wt], on_update=[])
                        out.append(nop)
                        changed = True
                    inst.sync_info = mybir.SyncInfo(
                        on_wait=[waits[-1]], on_update=list(si.on_update))
                out.append(inst)
            if changed:
                blk.instructions = out


def _build_nc(bias2_zero, bout_zero):
    f32 = mybir.dt.float32
    f32r = mybir.dt.float32r
    nc = bass.Bass()

    zg_e = nc.dram_tensor("zg_e", [IN_NC, 41, 128], f32r, kind="ExternalInput")
    zg_o = nc.dram_tensor("zg_o", [IN_NC, 41, 128], f32r, kind="ExternalInput")
    wl1_in = nc.dram_tensor("wl1_in", [128, 3, 128], f32r, kind="ExternalInput")
    wl2_in = nc.dram_tensor("wl2_in", [128, 3, 128], f32r, kind="ExternalInput")
    wl1_mid = nc.dram_tensor("wl1_mid", [NMID, 128, 3, 128], f32r,
                             kind="ExternalInput")
    wl2_mid = nc.dram_tensor("wl2_mid", [NMID, 128, 3, 128], f32r,
                             kind="ExternalInput")
    wout_lo = nc.dram_tensor("wout_lo", [128, NCH], f32r, kind="ExternalInput")
    wout_hi = nc.dram_tensor("wout_hi", [128, NCH], f32r, kind="ExternalInput")
    biases = nc.dram_tensor("biases", [NMID + 1, 128, 1], f32,
                            kind="ExternalInput")
    bout_r = nc.dram_tensor("bout_r", [1, NCH], f32r, kind="ExternalInput")
    ones_r = nc.dram_tensor("ones_r", [1, 128], f32r, kind="ExternalInput")
    # per-(channel,row) expanded KPN patch windows, per-partition contiguous
    # (882B per partition -> one descriptor each, 128 per DMA). bf16 for DVE
    # 2x-mode multiplies.
    bf16 = mybir.dt.bfloat16
    xw = nc.dram_tensor("xw", [IN_NC, 64, 128, KSIZE, KSIZE], bf16,
                        kind="ExternalInput")

    ydev = nc.dram_tensor("ydev", [128, IN_NC, 64], f32, kind="ExternalOutput")
    nsdev = nc.dram_tensor("nsdev", [128, 64], f32, kind="ExternalOutput")

    with tile.TileContext(nc) as tc:
        wpool = tc.alloc_tile_pool(name="w", bufs=1)
        gpool = tc.alloc_tile_pool(name="g", bufs=1)
        wmpool = tc.alloc_tile_pool(name="wmid", bufs=3)
        tpool = tc.alloc_tile_pool(name="rtmp", bufs=3)
        ppool = tc.alloc_tile_pool(name="patch", bufs=8)
        epool = tc.alloc_tile_pool(name="exp", bufs=3)
        spool = tc.alloc_tile_pool(name="small", bufs=4)
        psum = tc.alloc_tile_pool(name="ps", bufs=6, space="PSUM")
        psum_o = tc.alloc_tile_pool(name="pso", bufs=2, space="PSUM")

        l1_in = wpool.tile([128, 3, 128], f32r)
        l2_in = wpool.tile([128, 3, 128], f32r)
        wo_lo = wpool.tile([128, NCH], f32r)
        wo_hi = wpool.tile([128, NCH], f32r)
        bias_t = wpool.tile([128, NMID + 1], f32)
        bo_t = wpool.tile([1, NCH], f32r)
        ones_t = wpool.tile([1, 128], f32r)
        nc.sync.dma_start(out=l1_in, in_=wl1_in[:])
        nc.sync.dma_start(out=l2_in, in_=wl2_in[:])
        nc.sync.dma_start(out=wo_lo, in_=wout_lo[:])
        nc.sync.dma_start(out=wo_hi, in_=wout_hi[:])
        nc.sync.dma_start(out=bias_t,
                          in_=biases[:].rearrange("l p one -> p (l one)"))
        nc.sync.dma_start(out=bo_t, in_=bout_r[:])
        nc.sync.dma_start(out=ones_t, in_=ones_r[:])

        g_z = gpool.tile([128, J, X], f32r)
        feat = gpool.tile([128, J, X], f32r)
        t1 = gpool.tile([128, J, X], f32r)
        nc.vector.memset(g_z[:].bitcast(mybir.dt.float32), 0.0)
        nc.vector.memset(feat[:].bitcast(mybir.dt.float32), 0.0)
        nc.vector.memset(t1[:].bitcast(mybir.dt.float32), 0.0)

        # z rows (shard-local, 0..80): even row r -> partitions 0:3 slot
        # r//2+1; odd row r -> partitions 64:67 slot (r+1)//2+1.
        # Host packs zg_e = even rows 0..80 (slots 1..41), zg_o = odd rows
        # 1..79 at slots 2..41 (zg_o[0] stays zero -> slot 1 zero = row -1).
        nc.sync.dma_start(out=g_z[0:IN_NC, 1:42, 1:129], in_=zg_e[:])
        nc.sync.dma_start(out=g_z[64:64 + IN_NC, 1:42, 1:129], in_=zg_o[:])

        relu = mybir.ActivationFunctionType.Relu
        ident = mybir.ActivationFunctionType.Identity

        def conv(src, dst, l1, l2, bias_col, func, k_halo, residual):
            # output region: shard-local rows 0 .. 63 + k_halo -> slots 1..hi
            hi = (64 + k_halo) // 2 + 1      # top slot of even output rows
            blocks = [(s, min(4, hi - s + 1)) for s in range(1, hi + 1, 4)]
            # weight-major inside groups of 5 blocks: consecutive matmuls
            # share the stationary operand so walrus ldw-opt dedups the
            # (serialized, non-overlapping) LDWEIGHTS streams.
            for g0 in range(0, len(blocks), 5):
                grp = blocks[g0:g0 + 5]
                tiles = [psum.tile([128, 4, 128], f32, tag="convps",
                                   name=f"cps_{g0}_{i}")
                         for i in range(len(grp))]
                for wi in range(6):
                    dx, phase = wi % 3, wi // 3
                    wt = (l1 if phase == 0 else l2)[:, dx]
                    for (s0, mc), P in zip(grp, tiles):
                        o = s0 + phase
                        nc.tensor.matmul(
                            P[:, 0:mc], wt,
                            src[0:128, o:o + mc, dx:dx + 128],
                            start=(wi == 0), stop=(wi == 5))
                for (s0, mc), P in zip(grp, tiles):
                    if residual is None:
                        nc.scalar.activation(
                            out=dst[0:64, s0:s0 + mc, 1:129],
                            in_=P[0:64, 0:mc],
                            func=func, bias=bias_col[0:64], scale=1.0)
                        nc.scalar.activation(
                            out=dst[64:128, s0 + 1:s0 + 1 + mc, 1:129],
                            in_=P[64:128, 0:mc],
                            func=func, bias=bias_col[64:128], scale=1.0)
                    else:
                        # evacuate via ACT (bias folded), accumulate the
                        # residual on GpSimd (SBUF-only engine, otherwise
                        # idle) to keep DVE off the critical path
                        tmp = tpool.tile([128, 4, 128], f32, tag="rtmp")
                        nc.scalar.activation(
                            out=tmp[:, 0:mc], in_=P[:, 0:mc], func=ident,
                            bias=0.0, scale=1.0)
                        if not bias2_zero:
                            nc.vector.tensor_scalar(
                                out=tmp[:, 0:mc], in0=tmp[:, 0:mc],
                                scalar1=bias_col, scalar2=None,
                                op0=mybir.AluOpType.add)
                        nc.vector.tensor_add(
                            out=dst[0:64, s0:s0 + mc, 1:129],
                            in0=tmp[0:64, 0:mc],
                            in1=residual[0:64, s0:s0 + mc, 1:129])
                        nc.vector.tensor_add(
                            out=dst[64:128, s0 + 1:s0 + 1 + mc, 1:129],
                            in0=tmp[64:128, 0:mc],
                            in1=residual[64:128, s0 + 1:s0 + 1 + mc, 1:129])

        conv(g_z, feat, l1_in, l2_in, bias_t[:, 0:1], ident, 16, None)
        for rb in range(NB):
            la, lb = 2 * rb, 2 * rb + 1
            w1a = wmpool.tile([128, 3, 128], f32r, tag="w1")
            w2a = wmpool.tile([128, 3, 128], f32r, tag="w2")
            nc.sync.dma_start(out=w1a, in_=wl1_mid[la])
            nc.sync.dma_start(out=w2a, in_=wl2_mid[la])
            conv(feat, t1, w1a, w2a,
                 bias_t[:, 1 + la:2 + la], relu, 15 - 2 * rb, None)
            w1b = wmpool.tile([128, 3, 128], f32r, tag="w1")
            w2b = wmpool.tile([128, 3, 128], f32r, tag="w2")
            nc.sync.dma_start(out=w1b, in_=wl1_mid[lb])
            nc.sync.dma_start(out=w2b, in_=wl2_mid[lb])
            conv(t1, feat, w1b, w2b,
                 bias_t[:, 1 + lb:2 + lb], ident, 14 - 2 * rb, feat)

        yacc = spool.tile([128, IN_NC, 64], f32, tag="yacc")
        nsacc = spool.tile([128, 64], f32, tag="nsacc")

        for yl in range(64):
            if yl % 2 == 0:
                slot, wsel = yl // 2 + 1, wo_lo
            else:
                slot, wsel = (yl + 1) // 2 + 1, wo_hi
            Po = psum_o.tile([128, NCH], f32, tag="pout")
            nc.tensor.matmul(Po, feat[:, slot, 1:129], wsel,
                             start=True, stop=bout_zero)
            if not bout_zero:
                nc.tensor.matmul(Po, ones_t, bo_t, start=False, stop=True)
            ex = epool.tile([128, NCH], bf16, tag="ex")
            nc.scalar.activation(out=ex, in_=Po,
                                 func=mybir.ActivationFunctionType.Exp,
                                 scale=1.0)
            ssum = spool.tile([128, 1], f32, tag="ssum")
            nc.vector.reduce_sum(out=ssum, in_=ex, axis=mybir.AxisListType.X)
            rcp = spool.tile([128, 1], f32, tag="rcp")
            nc.vector.reciprocal(out=rcp, in_=ssum)
            exv = ex[:, 0:KSIZE * KSIZE].rearrange(
                "p (a b) -> p a b", a=KSIZE)
            dma_engines = [nc.sync, nc.sync, nc.sync]
            for c in range(IN_NC):
                patch = ppool.tile([128, KSIZE, KSIZE], bf16, tag="patch")
                dma_engines[c].dma_start(out=patch, in_=xw[c, yl])
                prod = epool.tile([128, KSIZE, KSIZE], bf16, tag="prod")
                pc = spool.tile([128, 1], f32, tag="pc")
                nc.vector.tensor_mul(out=prod, in0=exv, in1=patch)
                nc.vector.reduce_sum(
                    out=pc, in_=prod.rearrange("p a b -> p (a b)"),
                    axis=mybir.AxisListType.X)
                nc.vector.tensor_mul(out=yacc[:, c, yl:yl + 1], in0=pc,
                                     in1=rcp)
            nc.vector.tensor_mul(out=nsacc[:, yl:yl + 1],
                                 in0=ex[:, NCH - 1:NCH], in1=rcp)

        nc.sync.dma_start(out=ydev[:], in_=yacc)
        nc.sync.dma_start(out=nsdev[:], in_=nsacc)

        for p in (psum_o, psum, spool, epool, ppool, tpool, wmpool, gpool,
                  wpool):
            p.release()

    _legalize_waits(nc)
    return nc


def _stack_l1l2(Wl):
    # Wl [64o, ic, 3, 3] -> L1, L2 [128, 3, 128]
    ic = Wl.shape[1]
    L1 = np.zeros((128, 3, 128), np.float32)
    L2 = np.zeros((128, 3, 128), np.float32)
    for dx in range(3):
        L1[0:ic, dx, 0:64] = Wl[:, :, 1, dx].T
        L1[64:64 + ic, dx, 0:64] = Wl[:, :, 0, dx].T
        L1[0:ic, dx, 64:128] = Wl[:, :, 0, dx].T
        L2[64:64 + ic, dx, 0:64] = Wl[:, :, 2, dx].T
        L2[0:ic, dx, 64:128] = Wl[:, :, 2, dx].T
        L2[64:64 + ic, dx, 64:128] = Wl[:, :, 1, dx].T
    return L1, L2


def _prep_weights(w_in, w1s, w2s, w_out, flip):
    if flip:
        w_in = w_in[:, :, ::-1, :]
        w1s = w1s[:, :, :, ::-1, :]
        w2s = w2s[:, :, :, ::-1, :]
    l1_in, l2_in = _stack_l1l2(w_in)
    L1m = np.zeros((NMID, 128, 3, 128), np.float32)
    L2m = np.zeros((NMID, 128, 3, 128), np.float32)
    for rb in range(NB):
        L1m[2 * rb], L2m[2 * rb] = _stack_l1l2(w1s[rb])
        L1m[2 * rb + 1], L2m[2 * rb + 1] = _stack_l1l2(w2s[rb])
    wo = w_out[:, :, 0, 0]  # [442, 64]
    wlo = np.zeros((128, NCH), np.float32)
    whi = np.zeros((128, NCH), np.float32)
    wlo[0:64] = wo.T
    whi[64:128] = wo.T
    return l1_in, l2_in, L1m, L2m, wlo, whi


def kernel(x, z, eps, w_in, b_in, w1s, b1s, w2s, b2s, w_out, b_out):
    x = np.ascontiguousarray(np.asarray(x, np.float32))
    z = np.asarray(z, np.float32)
    eps = np.asarray(eps, np.float32)
    w_in = np.asarray(w_in, np.float32)
    b_in = np.asarray(b_in, np.float32)
    w1s = np.asarray(w1s, np.float32)
    b1s = np.asarray(b1s, np.float32)
    w2s = np.asarray(w2s, np.float32)
    b2s = np.asarray(b2s, np.float32)
    w_out = np.asarray(w_out, np.float32)
    b_out = np.asarray(b_out, np.float32)

    bias2_zero = bool(np.all(b2s == 0))
    bout_zero = bool(np.all(b_out == 0))
    _enable_ldw_opt()
    key = (bias2_zero, bout_zero)
    if key not in _cache:
        _cache[key] = _build_nc(bias2_zero, bout_zero)
    nc = _cache[key]

    weights = {}
    for flip in (False, True):
        l1_in, l2_in, L1m, L2m, wlo, whi = _prep_weights(
            w_in, w1s, w2s, w_out, flip)
        weights[flip] = (l1_in, l2_in, L1m, L2m, wlo, whi)

    biases = np.zeros((NMID + 1, 128, 1), np.float32)
    biases[0, 0:64, 0] = b_in
    biases[0, 64:128, 0] = b_in
    for rb in range(NB):
        biases[1 + 2 * rb, 0:64, 0] = b1s[rb]
        biases[1 + 2 * rb, 64:128, 0] = b1s[rb]
        biases[2 + 2 * rb, 0:64, 0] = b2s[rb]
        biases[2 + 2 * rb, 64:128, 0] = b2s[rb]
    bout_row = np.ascontiguousarray(b_out.reshape(1, NCH))
    ones_row = np.ones((1, 128), np.float32)

    # padded x (vertical dim only logical; we slice rows directly)
    in_maps = []
    for core in range(N_CORES):
        b, half = core // 2, core % 2
        flip = half == 1
        # shard-local z rows 0..80: top zl[r] = z[b, r]; bottom z flipped
        zl = z[b] if not flip else z[b, :, ::-1]
        zg_e = np.zeros((IN_NC, 41, 128), np.float32)
        zg_o = np.zeros((IN_NC, 41, 128), np.float32)
        zg_e[:, 0:41] = zl[:, 0:81:2]          # rows 0,2,..,80 -> slots 1..41
        zg_o[:, 1:41] = zl[:, 1:80:2]          # rows 1,3,..,79 -> slots 2..41
        # KPN patch windows, fully expanded per output row:
        # xw[c, yl, x0, t, u] = xp[c, 4*y0(yl) + t, 4*x0 + u] with
        # y0 = yl (top) or 127 - yl (bottom flipped), xp = x padded by 10.
        import ml_dtypes
        xp = np.zeros((IN_NC, H + 2 * 10, W + 2 * 10), dtype=ml_dtypes.bfloat16)
        xp[:, 10:10 + H, 10:10 + W] = x[b]
        y0s = np.arange(64) if not flip else (127 - np.arange(64))
        ridx = (4 * y0s)[:, None] + np.arange(KSIZE)[None, :]   # [64, 21]
        cols = 4 * np.arange(128)[:, None] + np.arange(KSIZE)[None, :]
        sub = xp[:, ridx]                 # [3, 64, 21, 532]
        sub = sub[:, :, :, cols]          # [3, 64, 21, 128, 21]
        xw_arr = np.ascontiguousarray(np.transpose(sub, (0, 1, 3, 2, 4)))
        l1_in, l2_in, L1m, L2m, wlo, whi = weights[flip]
        in_maps.append({
            "zg_e": zg_e, "zg_o": zg_o,
            "wl1_in": l1_in, "wl2_in": l2_in,
            "wl1_mid": L1m, "wl2_mid": L2m,
            "wout_lo": wlo, "wout_hi": whi,
            "biases": biases, "bout_r": bout_row, "ones_r": ones_row,
            "xw": xw_arr,
        })

    trace = bool(globals().get("TRACE", False))
    res = run_bass_kernel_spmd(nc, in_maps, core_ids=list(range(N_CORES)),
                               trace=trace)
    globals()["_last_result"] = res

    out = np.zeros((B, IN_NC, h, w), np.float32)
    for bb in range(B):
        ns_sum = (float(res.results[2 * bb]["nsdev"].sum())
                  + float(res.results[2 * bb + 1]["nsdev"].sum()))
        mean_ns = ns_sum / (h * w)
        for half in range(2):
            ydev = res.results[2 * bb + half]["ydev"]  # [128, 3, 64]
            yt = np.transpose(ydev, (1, 2, 0))         # [3, 64, 128]
            if half == 0:
                out[bb, :, 0:64, :] = yt
            else:
                out[bb, :, 64:128, :] = yt[:, ::-1, :]
        out[bb] += mean_ns * eps[bb]
    return out


# revision 15
# speedup vs baseline: 1.2543x; 1.0716x over previous
"""Trainium2 Bass kernel for nn_DegModel (EDSR-style degradation backbone +
per-pixel KPN), distributed over 8 NeuronCores.

Sharding: one core per (batch, image-half): core i -> batch i//2, half i%2.
Each core runs the whole backbone locally on its 64-row half plus a 17-row
recomputed halo, so no collectives are needed. Bottom halves are processed
vertically flipped (host flips z and the dy axis of the conv weights, both
per-core input data), which makes the on-device geometry identical for all
cores. The only cross-core quantity — the global mean of the predicted noise
channel — is reduced on host from per-core partial sums.

Feature maps live in SBUF as [128 partitions, J slots, 130] with partition
p = channel + 64*parity and the odd-row half skewed one slot down:
lower[c, j] = F[c, 2j], upper[c, j] = F[c, 2j-1]. With this skew a 3x3 conv
over an 8-row output block is exactly 6 full K=128 x M=128 float32r matmuls
(2 per kernel column dx) into one [128, 4, 128] PSUM bank: M columns 0:64
produce the even output rows, 64:128 the odd rows.

conv_out (1x1) + softmax + the 21x21 KPN run per output row in
pixel-partition layout: feat[:, slot, 1:129] is the stationary operand
(M = 128 pixels) against moving w_out [128, 442], landing logits as
[pixel, channel]; exp / reduce / divide are then free-dim ops, and the
softmax normalization is folded to after the KPN sum
(y = sum(patch * exp) / sum(exp)).
"""

import sys

sys.path.insert(0, "/opt/trn_rl_repo")

import numpy as np

import concourse.bass as bass
import concourse.tile as tile
from concourse import mybir
from concourse.bass_utils import run_bass_kernel_spmd

KSIZE = 21
NF = 64
NB = 8
IN_NC = 3
B, H, W = 4, 512, 512
h = w = 128
NCH = KSIZE * KSIZE + 1  # 442

N_CORES = 8
J = 44    # feature-buffer slots (2 image rows per slot)
X = 130   # 128 cols + 2 zero pad cols
NMID = 2 * NB
XPW = 532  # padded x row length (512 + 2*10)

_cache = {}


def _enable_ldw_opt():
    import concourse.bass_utils as _bu
    if getattr(_bu, "_ldw_opt_patched", False):
        return
    _orig = _bu.run_command

    def _patched(cmd, **kw):
        if isinstance(cmd, list):
            cmd = ["--enable-ldw-opt=true" if c == "--enable-ldw-opt=false"
                   else c for c in cmd]
        return _orig(cmd, **kw)

    _bu.run_command = _patched
    _bu._ldw_opt_patched = True


def _legalize_waits(nc):
    """This walrus build rejects >1 sync wait per instruction; move extra
    waits onto same-engine NOPs inserted immediately before (engines are
    in-order, so semantics are preserved)."""
    for fn in nc.m.functions:
        for blk in fn.blocks:
            out, changed = [], False
            for inst in blk.instructions:
                si = inst.sync_info
                if si is not None and len(si.on_wait) > 1:
                    waits = list(si.on_wait)
                    for wt in waits[:-1]:
                        nop = mybir.InstNoOp(
                            name=nc.get_next_instruction_name(),
                            ins=[], outs=[], engine=inst.engine)
                        nop.sync_info = mybir.SyncInfo(on_wait=[wt], on_update=[])
                        out.append(nop)
                        changed = True
                    inst.sync_info = mybir.SyncInfo(
                        on_wait=[waits[-1]], on_update=list(si.on_update))
                out.append(inst)
            if changed:
                blk.instructions = out


def _build_nc(bias2_zero, bout_zero):
    f32 = mybir.dt.float32
    f32r = mybir.dt.float32r
    nc = bass.Bass()

    zg_e = nc.dram_tensor("zg_e", [IN_NC, 41, 128], f32r, kind="ExternalInput")
    zg_o = nc.dram_tensor("zg_o", [IN_NC, 41, 128], f32r, kind="ExternalInput")
    wl1_in = nc.dram_tensor("wl1_in", [128, 3, 128], f32r, kind="ExternalInput")
    wl2_in = nc.dram_tensor("wl2_in", [128, 3, 128], f32r, kind="ExternalInput")
    wl1_mid = nc.dram_tensor("wl1_mid", [NMID, 128, 3, 128], f32r,
                             kind="ExternalInput")
    wl2_mid = nc.dram_tensor("wl2_mid", [NMID, 128, 3, 128], f32r,
                             kind="ExternalInput")
    wout_lo = nc.dram_tensor("wout_lo", [128, NCH], f32r, kind="ExternalInput")
    wout_hi = nc.dram_tensor("wout_hi", [128, NCH], f32r, kind="ExternalInput")
    biases = nc.dram_tensor("biases", [NMID + 1, 128, 1], f32,
                            kind="ExternalInput")
    bout_r = nc.dram_tensor("bout_r", [1, NCH], f32r, kind="ExternalInput")
    ones_r = nc.dram_tensor("ones_r", [1, 128], f32r, kind="ExternalInput")
    # per-(channel,row) expanded KPN patch windows, per-partition contiguous
    # (882B per partition -> one descriptor each, 128 per DMA). bf16 for DVE
    # 2x-mode multiplies.
    bf16 = mybir.dt.bfloat16
    xw = nc.dram_tensor("xw", [IN_NC, 64, 128, KSIZE, KSIZE], bf16,
                        kind="ExternalInput")

    ydev = nc.dram_tensor("ydev", [128, IN_NC, 64], f32, kind="ExternalOutput")
    nsdev = nc.dram_tensor("nsdev", [128, 64], f32, kind="ExternalOutput")

    with tile.TileContext(nc) as tc:
        wpool = tc.alloc_tile_pool(name="w", bufs=1)
        gpool = tc.alloc_tile_pool(name="g", bufs=1)
        wmpool = tc.alloc_tile_pool(name="wmid", bufs=3)
        tpool = tc.alloc_tile_pool(name="rtmp", bufs=3)
        ppool = tc.alloc_tile_pool(name="patch", bufs=8)
        epool = tc.alloc_tile_pool(name="exp", bufs=3)
        spool = tc.alloc_tile_pool(name="small", bufs=4)
        psum = tc.alloc_tile_pool(name="ps", bufs=6, space="PSUM")
        psum_o = tc.alloc_tile_pool(name="pso", bufs=2, space="PSUM")

        l1_in = wpool.tile([128, 3, 128], f32r)
        l2_in = wpool.tile([128, 3, 128], f32r)
        wo_lo = wpool.tile([128, NCH], f32r)
        wo_hi = wpool.tile([128, NCH], f32r)
        bias_t = wpool.tile([128, NMID + 1], f32)
        bo_t = wpool.tile([1, NCH], f32r)
        ones_t = wpool.tile([1, 128], f32r)
        nc.sync.dma_start(out=l1_in, in_=wl1_in[:])
        nc.sync.dma_start(out=l2_in, in_=wl2_in[:])
        nc.sync.dma_start(out=wo_lo, in_=wout_lo[:])
        nc.sync.dma_start(out=wo_hi, in_=wout_hi[:])
        nc.sync.dma_start(out=bias_t,
                          in_=biases[:].rearrange("l p one -> p (l one)"))
        nc.sync.dma_start(out=bo_t, in_=bout_r[:])
        nc.sync.dma_start(out=ones_t, in_=ones_r[:])

        g_z = gpool.tile([128, J, X], f32r)
        feat = gpool.tile([128, J, X], f32r)
        t1 = gpool.tile([128, J, X], f32r)
        nc.vector.memset(g_z[:].bitcast(mybir.dt.float32), 0.0)
        nc.vector.memset(feat[:].bitcast(mybir.dt.float32), 0.0)
        nc.vector.memset(t1[:].bitcast(mybir.dt.float32), 0.0)

        # z rows (shard-local, 0..80): even row r -> partitions 0:3 slot
        # r//2+1; odd row r -> partitions 64:67 slot (r+1)//2+1.
        # Host packs zg_e = even rows 0..80 (slots 1..41), zg_o = odd rows
        # 1..79 at slots 2..41 (zg_o[0] stays zero -> slot 1 zero = row -1).
        nc.sync.dma_start(out=g_z[0:IN_NC, 1:42, 1:129], in_=zg_e[:])
        nc.sync.dma_start(out=g_z[64:64 + IN_NC, 1:42, 1:129], in_=zg_o[:])

        relu = mybir.ActivationFunctionType.Relu
        ident = mybir.ActivationFunctionType.Identity

        def conv(src, dst, l1, l2, bias_col, func, k_halo, residual):
            # output region: shard-local rows 0 .. 63 + k_halo -> slots 1..hi
            hi = (64 + k_halo) // 2 + 1      # top slot of even output rows
            blocks = [(s, min(4, hi - s + 1)) for s in range(1, hi + 1, 4)]
            # weight-major inside groups of 5 blocks: consecutive matmuls
            # share the stationary operand so walrus ldw-opt dedups the
            # (serialized, non-overlapping) LDWEIGHTS streams.
            for g0 in range(0, len(blocks), 5):
                grp = blocks[g0:g0 + 5]
                tiles = [psum.tile([128, 4, 128], f32, tag="convps",
                                   name=f"cps_{g0}_{i}")
                         for i in range(len(grp))]
                for wi in range(6):
                    dx, phase = wi % 3, wi // 3
                    wt = (l1 if phase == 0 else l2)[:, dx]
                    for (s0, mc), P in zip(grp, tiles):
                        o = s0 + phase
                        nc.tensor.matmul(
                            P[:, 0:mc], wt,
                            src[0:128, o:o + mc, dx:dx + 128],
                            start=(wi == 0), stop=(wi == 5))
                for (s0, mc), P in zip(grp, tiles):
                    if residual is None:
                        nc.scalar.activation(
                            out=dst[0:64, s0:s0 + mc, 1:129],
                            in_=P[0:64, 0:mc],
                            func=func, bias=bias_col[0:64], scale=1.0)
                        nc.scalar.activation(
                            out=dst[64:128, s0 + 1:s0 + 1 + mc, 1:129],
                            in_=P[64:128, 0:mc],
                            func=func, bias=bias_col[64:128], scale=1.0)
                    else:
                        # evacuate via ACT (bias folded), accumulate the
                        # residual on GpSimd (SBUF-only engine, otherwise
                        # idle) to keep DVE off the critical path
                        tmp = tpool.tile([128, 4, 128], f32, tag="rtmp")
                        nc.scalar.activation(
                            out=tmp[:, 0:mc], in_=P[:, 0:mc], func=ident,
                            bias=0.0, scale=1.0)
                        if not bias2_zero:
                            nc.vector.tensor_scalar(
                                out=tmp[:, 0:mc], in0=tmp[:, 0:mc],
                                scalar1=bias_col, scalar2=None,
                                op0=mybir.AluOpType.add)
                        nc.vector.tensor_add(
                            out=dst[0:64, s0:s0 + mc, 1:129],
                            in0=tmp[0:64, 0:mc],
                            in1=residual[0:64, s0:s0 + mc, 1:129])
                        nc.vector.tensor_add(
                            out=dst[64:128, s0 + 1:s0 + 1 + mc, 1:129],
                            in0=tmp[64:128, 0:mc],
                            in1=residual[64:128, s0 + 1:s0 + 1 + mc, 1:129])

        conv(g_z, feat, l1_in, l2_in, bias_t[:, 0:1], ident, 16, None)
        for rb in range(NB):
            la, lb = 2 * rb, 2 * rb + 1
            w1a = wmpool.tile([128, 3, 128], f32r, tag="w1")
            w2a = wmpool.tile([128, 3, 128], f32r, tag="w2")
            nc.sync.dma_start(out=w1a, in_=wl1_mid[la])
            nc.sync.dma_start(out=w2a, in_=wl2_mid[la])
            conv(feat, t1, w1a, w2a,
                 bias_t[:, 1 + la:2 + la], relu, 15 - 2 * rb, None)
            w1b = wmpool.tile([128, 3, 128], f32r, tag="w1")
            w2b = wmpool.tile([128, 3, 128], f32r, tag="w2")
            nc.sync.dma_start(out=w1b, in_=wl1_mid[lb])
            nc.sync.dma_start(out=w2b, in_=wl2_mid[lb])
            conv(t1, feat, w1b, w2b,
                 bias_t[:, 1 + lb:2 + lb], ident, 14 - 2 * rb, feat)

        yacc = spool.tile([128, IN_NC, 64], f32, tag="yacc")
        nsacc = spool.tile([128, 64], f32, tag="nsacc")

        for yl in range(64):
            if yl % 2 == 0:
                slot, wsel = yl // 2 + 1, wo_lo
            else:
                slot, wsel = (yl + 1) // 2 + 1, wo_hi
            Po = psum_o.tile([128, NCH], f32, tag="pout")
            nc.tensor.matmul(Po, feat[:, slot, 1:129], wsel,
                             start=True, stop=bout_zero)
            if not bout_zero:
                nc.tensor.matmul(Po, ones_t, bo_t, start=False, stop=True)
            ex = epool.tile([128, NCH], bf16, tag="ex")
            ssum = spool.tile([128, 1], f32, tag="ssum")
            nc.scalar.activation(out=ex, in_=Po,
                                 func=mybir.ActivationFunctionType.Exp,
                                 scale=1.0, accum_out=ssum)
            rcp = spool.tile([128, 1], f32, tag="rcp")
            nc.vector.reciprocal(out=rcp, in_=ssum)
            exf = ex[:, 0:KSIZE * KSIZE]
            dma_engines = [nc.sync, nc.scalar, nc.gpsimd]
            for c in range(IN_NC):
                patch = ppool.tile([128, KSIZE * KSIZE], bf16, tag="patch")
                dma_engines[c].dma_start(
                    out=patch.rearrange("p (a b) -> p a b", a=KSIZE),
                    in_=xw[c, yl])
                prod = epool.tile([128, KSIZE * KSIZE], bf16, tag="prod")
                pc = spool.tile([128, 1], f32, tag="pc")
                nc.vector.tensor_mul(out=prod, in0=exf, in1=patch)
                nc.vector.reduce_sum(out=pc, in_=prod,
                                     axis=mybir.AxisListType.X)
                nc.vector.tensor_mul(out=yacc[:, c, yl:yl + 1], in0=pc,
                                     in1=rcp)
            nc.vector.tensor_mul(out=nsacc[:, yl:yl + 1],
                                 in0=ex[:, NCH - 1:NCH], in1=rcp)

        nc.sync.dma_start(out=ydev[:], in_=yacc)
        nc.sync.dma_start(out=nsdev[:], in_=nsacc)

        for p in (psum_o, psum, spool, epool, ppool, tpool, wmpool, gpool,
                  wpool):
            p.release()

    _legalize_waits(nc)
    return nc


def _stack_l1l2(Wl):
    # Wl [64o, ic, 3, 3] -> L1, L2 [128, 3, 128]
    ic = Wl.shape[1]
    L1 = np.zeros((128, 3, 128), np.float32)
    L2 = np.zeros((128, 3, 128), np.float32)
    for dx in range(3):
        L1[0:ic, dx, 0:64] = Wl[:, :, 1, dx].T
        L1[64:64 + ic, dx, 0:64] = Wl[:, :, 0, dx].T
        L1[0:ic, dx, 64:128] = Wl[:, :, 0, dx].T
        L2[64:64 + ic, dx, 0:64] = Wl[:, :, 2, dx].T
        L2[0:ic, dx, 64:128] = Wl[:, :, 2, dx].T
        L2[64:64 + ic, dx, 64:128] = Wl[:, :, 1, dx].T
    return L1, L2


def _prep_weights(w_in, w1s, w2s, w_out, flip):
    if flip:
        w_in = w_in[:, :, ::-1, :]
        w1s = w1s[:, :, :, ::-1, :]
        w2s = w2s[:, :, :, ::-1, :]
    l1_in, l2_in = _stack_l1l2(w_in)
    L1m = np.zeros((NMID, 128, 3, 128), np.float32)
    L2m = np.zeros((NMID, 128, 3, 128), np.float32)
    for rb in range(NB):
        L1m[2 * rb], L2m[2 * rb] = _stack_l1l2(w1s[rb])
        L1m[2 * rb + 1], L2m[2 * rb + 1] = _stack_l1l2(w2s[rb])
    wo = w_out[:, :, 0, 0]  # [442, 64]
    wlo = np.zeros((128, NCH), np.float32)
    whi = np.zeros((128, NCH), np.float32)
    wlo[0:64] = wo.T
    whi[64:128] = wo.T
    return l1_in, l2_in, L1m, L2m, wlo, whi


def kernel(x, z, eps, w_in, b_in, w1s, b1s, w2s, b2s, w_out, b_out):
    x = np.ascontiguousarray(np.asarray(x, np.float32))
    z = np.asarray(z, np.float32)
    eps = np.asarray(eps, np.float32)
    w_in = np.asarray(w_in, np.float32)
    b_in = np.asarray(b_in, np.float32)
    w1s = np.asarray(w1s, np.float32)
    b1s = np.asarray(b1s, np.float32)
    w2s = np.asarray(w2s, np.float32)
    b2s = np.asarray(b2s, np.float32)
    w_out = np.asarray(w_out, np.float32)
    b_out = np.asarray(b_out, np.float32)

    bias2_zero = bool(np.all(b2s == 0))
    bout_zero = bool(np.all(b_out == 0))
    _enable_ldw_opt()
    key = (bias2_zero, bout_zero)
    if key not in _cache:
        _cache[key] = _build_nc(bias2_zero, bout_zero)
    nc = _cache[key]

    weights = {}
    for flip in (False, True):
        l1_in, l2_in, L1m, L2m, wlo, whi = _prep_weights(
            w_in, w1s, w2s, w_out, flip)
        weights[flip] = (l1_in, l2_in, L1m, L2m, wlo, whi)

    biases = np.zeros((NMID + 1, 128, 1), np.float32)
    biases[0, 0:64, 0] = b_in
    biases[0, 64:128, 0] = b_in
    for rb in range(NB):
        biases[1 + 2 * rb, 0:64, 0] = b1s[rb]
        biases[1 + 2 * rb, 64:128, 0] = b1s[rb]
        biases[2 + 2 * rb, 0:64, 0] = b2s[rb]
        biases[2 + 2 * rb, 64:128, 0] = b2s[rb]
    bout_row = np.ascontiguousarray(b_out.reshape(1, NCH))
    ones_row = np.ones((1, 128), np.float32)

    # padded x (vertical dim only logical; we slice rows directly)
    in_maps = []
    for core in range(N_CORES):
        b, half = core // 2, core % 2
        flip = half == 1
        # shard-local z rows 0..80: top zl[r] = z[b, r]; bottom z flipped
        zl = z[b] if not flip else z[b, :, ::-1]
        zg_e = np.zeros((IN_NC, 41, 128), np.float32)
        zg_o = np.zeros((IN_NC, 41, 128), np.float32)
        zg_e[:, 0:41] = zl[:, 0:81:2]          # rows 0,2,..,80 -> slots 1..41
        zg_o[:, 1:41] = zl[:, 1:80:2]          # rows 1,3,..,79 -> slots 2..41
        # KPN patch windows, fully expanded per output row:
        # xw[c, yl, x0, t, u] = xp[c, 4*y0(yl) + t, 4*x0 + u] with
        # y0 = yl (top) or 127 - yl (bottom flipped), xp = x padded by 10.
        import ml_dtypes
        xp = np.zeros((IN_NC, H + 2 * 10, W + 2 * 10), dtype=ml_dtypes.bfloat16)
        xp[:, 10:10 + H, 10:10 + W] = x[b]
        y0s = np.arange(64) if not flip else (127 - np.arange(64))
        ridx = (4 * y0s)[:, None] + np.arange(KSIZE)[None, :]   # [64, 21]
        cols = 4 * np.arange(128)[:, None] + np.arange(KSIZE)[None, :]
        sub = xp[:, ridx]                 # [3, 64, 21, 532]
        sub = sub[:, :, :, cols]          # [3, 64, 21, 128, 21]
        xw_arr = np.ascontiguousarray(np.transpose(sub, (0, 1, 3, 2, 4)))
        l1_in, l2_in, L1m, L2m, wlo, whi = weights[flip]
        in_maps.append({
            "zg_e": zg_e, "zg_o": zg_o,
            "wl1_in": l1_in, "wl2_in": l2_in,
            "wl1_mid": L1m, "wl2_mid": L2m,
            "wout_lo": wlo, "wout_hi": whi,
            "biases": biases, "bout_r": bout_row, "ones_r": ones_row,
            "xw": xw_arr,
        })

    trace = bool(globals().get("TRACE", False))
    res = run_bass_kernel_spmd(nc, in_maps, core_ids=list(range(N_CORES)),
                               trace=trace)
    globals()["_last_result"] = res

    out = np.zeros((B, IN_NC, h, w), np.float32)
    for bb in range(B):
        ns_sum = (float(res.results[2 * bb]["nsdev"].sum())
                  + float(res.results[2 * bb + 1]["nsdev"].sum()))
        mean_ns = ns_sum / (h * w)
        for half in range(2):
            ydev = res.results[2 * bb + half]["ydev"]  # [128, 3, 64]
            yt = np.transpose(ydev, (1, 2, 0))         # [3, 64, 128]
            if half == 0:
                out[bb, :, 0:64, :] = yt
            else:
                out[bb, :, 64:128, :] = yt[:, ::-1, :]
        out[bb] += mean_ns * eps[bb]
    return out
